# revision 5
# baseline (speedup 1.0000x reference)
"""Multi-head attention (B=2, N=4096, C=512, H=8, D=64) on 8 TRN2 NeuronCores.

Sharding: core c handles batch b = c // 4 and head-pair p = c % 4
(heads 2p, 2p+1, i.e. channels [128p, 128p+128) of the QKV projections).
Each core computes a partial output projection O_loc @ Wo_loc; the host
sums the 4 partials per batch and adds bo. No collectives needed.

v2 over the 334us baseline (which was ScalarE-exp-bound at ~294us busy):
  - The exp is split between ScalarE (exact LUT exp on S cols [0:ACOL])
    and VectorE (cols [ACOL:1024] via a single tensor_scalar:
    y = s*23.083 + 16248.7 converted to int16, whose bit pattern IS
    bf16(exp(s/8)) by the Schraudolph trick; ~1.8% per-element RMS error
    that washes out to ~1e-3 end-to-end after the softmax ratio).
  - The PV matmuls run one kc-step behind the exp so the PE's in-order
    queue never waits on ACT/DVE within a step.
  - PSUM (8 banks): S double-buffer [128,1024]x2 (4) + single PV
    accumulator [128,1024] (2) + proj pool [128,512]x2 (2).  The PV
    pairs of each q-block's first steps are deferred until the previous
    block's epilogue TTs free the PV banks, then caught up 2/step.
  - Epilogue per q-block: reciprocal_approx_fast on the PSUM denominator
    row, one gpsimd partition_broadcast, two fused normalize-TTs reading
    PV PSUM directly (no o2tu staging), out-proj chunks with ScalarE
    PSUM->SBUF copies (ACT has slack), DMA per 128-row chunk.
"""
import numpy as np
import ml_dtypes

import concourse.bass as bass
import concourse.mybir as mybir
import concourse.tile as tile
from concourse.tile_rust import add_dep_helper
from concourse import bacc
from concourse.bass_utils import run_bass_kernel_spmd

F32 = mybir.dt.float32
BF16 = mybir.dt.bfloat16
I16 = mybir.dt.int16
AF = mybir.ActivationFunctionType
ALU = mybir.AluOpType

N = 4096
C = 512
HD = 128          # channels per core (2 heads x 64)
D = 64
QB = 512          # q-block
NQB = N // QB     # 8
KC = 128          # key chunk
NKC = N // KC     # 32
VSTR = 65         # [V(64) | ones] stride inside vaug
VOFF = 2144       # head offset inside the combined vaug tile

ACOL = 544        # exp columns done by ScalarE; rest by VectorE
# Schraudolph: bits_bf16(exp(s/8)) ~= round(s * (2^7*log2e/8) + (127*2^7 - C))
EXP_MUL = (2.0 ** 7) * 1.4426950408889634 / 8.0    # 23.0831206...
EXP_ADD = 127.0 * 128.0 - 7.3                      # 16248.7


def build_nc(debug=False):
    nc = bacc.Bacc(None, target_bir_lowering=False)

    xT = nc.declare_dram_parameter("xT", [C, N], BF16, isOutput=False)
    wq = nc.declare_dram_parameter("wq", [C, HD], BF16, isOutput=False)
    wk = nc.declare_dram_parameter("wk", [C, HD], BF16, isOutput=False)
    wv = nc.declare_dram_parameter("wv", [C, HD], BF16, isOutput=False)
    wo = nc.declare_dram_parameter("wo", [HD, C], BF16, isOutput=False)
    bq = nc.declare_dram_parameter("bq", [HD, 1], F32, isOutput=False)
    bk = nc.declare_dram_parameter("bk", [HD, 1], F32, isOutput=False)
    bv = nc.declare_dram_parameter("bv", [1, HD], BF16, isOutput=False)
    out = nc.declare_dram_parameter("out", [N, C], F32, isOutput=True)
    dbg = {}
    if debug:
        dbg["kt"] = nc.declare_dram_parameter("d_kt", [HD, N], BF16, isOutput=True)
        dbg["qt"] = nc.declare_dram_parameter("d_qt", [HD, N], BF16, isOutput=True)
        dbg["va"] = nc.declare_dram_parameter("d_va", [128, 2 * VOFF], BF16, isOutput=True)
        dbg["p00"] = nc.declare_dram_parameter("d_p00", [128, 2 * QB], BF16, isOutput=True)
        dbg["pv0"] = nc.declare_dram_parameter("d_pv0", [128, 2 * QB], F32, isOutput=True)
        dbg["rcp0"] = nc.declare_dram_parameter("d_rcp0", [1, 2 * QB], F32, isOutput=True)
        dbg["o2t0"] = nc.declare_dram_parameter("d_o2t0", [HD, QB], BF16, isOutput=True)

    with tile.TileContext(nc) as tc:
        with (
            tc.tile_pool(name="const", bufs=1) as cpool,
            tc.tile_pool(name="big", bufs=1) as bpool,
        ):
            # Constants / weights in SBUF
            xt = [cpool.tile([128, N], BF16, tag=f"xt{c}", name=f"xt{c}") for c in range(4)]
            wq_s = cpool.tile([128, C], BF16, tag="wq")
            wk_s = cpool.tile([128, C], BF16, tag="wk")
            wv_s = cpool.tile([128, C], BF16, tag="wv")
            wo_s = cpool.tile([HD, C], BF16, tag="wo")
            bq_s = cpool.tile([HD, 1], F32, tag="bq")
            bk_s = cpool.tile([HD, 1], F32, tag="bk")
            bv_s = cpool.tile([1, HD], BF16, tag="bv")
            ones_s = cpool.tile([1, 128], BF16, tag="ones")

            # Critical-path-first DMA order (per-DMA first-byte latency is
            # ~1us, so keep the prefix short): K/Q weights as single strided
            # DMAs, then xT block 0, then everything else. Two DGE queues.
            dma_engines = [nc.sync, nc.gpsimd]
            wk_r = wk[:].rearrange("(c p) m -> p c m", p=128)
            wq_r = wq[:].rearrange("(c p) m -> p c m", p=128)
            wv_r = wv[:].rearrange("(c p) m -> p c m", p=128)
            nc.sync.dma_start(
                out=wk_s[:].rearrange("p (c m) -> p c m", c=4), in_=wk_r)
            nc.gpsimd.dma_start(
                out=wq_s[:].rearrange("p (c m) -> p c m", c=4), in_=wq_r)
            for c in range(4):
                # tiny prefix: lets a 128-position K projection (and so the
                # first S matmul) start ~10us earlier
                eng = dma_engines[c % 2]
                eng.dma_start(out=xt[c][:, 0:128],
                              in_=xT[c * 128:(c + 1) * 128, 0:128])
            for blk in range(NQB):
                bsl = (slice(128, QB) if blk == 0
                       else slice(blk * QB, (blk + 1) * QB))
                for c in range(4):
                    eng = dma_engines[(blk * 4 + c) % 2]
                    eng.dma_start(out=xt[c][:, bsl],
                                  in_=xT[c * 128:(c + 1) * 128, bsl])
                if blk == 0:
                    nc.sync.dma_start(out=bk_s[:], in_=bk[:])
                    nc.gpsimd.dma_start(out=bq_s[:], in_=bq[:])
                    nc.sync.dma_start(out=wv_s[:].rearrange("p (c m) -> p c m", c=4), in_=wv_r)
                    nc.gpsimd.dma_start(out=bv_s[:], in_=bv[:])
            nc.sync.dma_start(out=wo_s[:], in_=wo[:])
            nc.vector.memset(ones_s[:], 1.0)

            # Persistent activations
            qt = bpool.tile([HD, N], BF16, tag="qt")
            kt = bpool.tile([HD, N], BF16, tag="kt")
            # combined V_aug for both heads: head h at cols [h*VOFF, ...),
            # per-kc [64 V | 1 ones] chunks at stride 65; padded so the PV
            # stationary can always load a 128-col window.
            vaug = bpool.tile([128, 2 * VOFF], BF16, tag="vaug")
            nc.vector.memset(vaug[:], 1.0)

            # warm the ACT exp table early so the ~2.7us load overlaps DMA
            wrm = bpool.tile([1, 128], BF16, tag="wrm")
            nc.scalar.activation(wrm[:], ones_s[:], AF.Exp)

            with (
                tc.tile_pool(name="sps", bufs=2, space="PSUM") as sps,
                tc.tile_pool(name="pvp", bufs=1, space="PSUM") as pvp,
                tc.tile_pool(name="pjp", bufs=2, space="PSUM") as pjp,
                tc.tile_pool(name="ptp", bufs=6) as ptp,
                tc.tile_pool(name="msc", bufs=2) as msc,
                tc.tile_pool(name="o2p", bufs=2) as o2p,
                tc.tile_pool(name="obp", bufs=4) as obp,
            ):
                def proj_qk(which, qb, lo=0, hi=QB):
                    sl = slice(qb * QB + lo, qb * QB + hi)
                    w_s, b_s, dst = ((wq_s, bq_s, qt) if which == "q"
                                     else (wk_s, bk_s, kt))
                    pq = pjp.tile([128, QB], F32, tag="pj", name="pj")
                    for c in range(4):
                        nc.tensor.matmul(pq[:, 0:hi - lo],
                                         lhsT=w_s[:, c * 128:(c + 1) * 128],
                                         rhs=xt[c][:, sl],
                                         start=(c == 0), stop=(c == 3))
                    nc.vector.tensor_scalar(out=dst[:, sl], in0=pq[:, 0:hi - lo],
                                            scalar1=b_s[:], scalar2=None,
                                            op0=ALU.add)

                def proj_v(pt):
                    psl = slice(pt * 128, (pt + 1) * 128)
                    pv = pjp.tile([128, QB], F32, tag="pj", name="pj")
                    for c in range(4):
                        nc.tensor.matmul(pv[:, 0:128], lhsT=xt[c][:, psl],
                                         rhs=wv_s[:, c * 128:(c + 1) * 128],
                                         start=(c == 0), stop=False)
                    nc.tensor.matmul(pv[:, 0:128], lhsT=ones_s[:], rhs=bv_s[:],
                                     start=False, stop=True)
                    # one copy for both heads: [128, 2, 64] strided into vaug
                    dst = vaug[:].rearrange("p (h z) -> p h z", h=2)
                    dst = dst[:, :, pt * VSTR:pt * VSTR + 64]
                    src = pv[:, 0:128].rearrange("p (h z) -> p h z", z=64)
                    nc.vector.tensor_copy(out=dst, in_=src)

                # minimal prologue; the rest of the projections interleave
                # into qb=0's kc loop, keeping PE, ACT and DVE busy
                proj_qk("k", 0, 0, 128)
                proj_qk("q", 0)
                proj_qk("k", 0, 128, QB)
                proj_v(0)
                proj_v(1)
                pending_proj = []
                for j in range(1, 8):
                    pending_proj.append(("v", j + 1))
                    pending_proj.append(("k", j))
                for pt in range(9, NKC):
                    pending_proj.append(("v", pt))
                pending_proj.reverse()  # pop() from the front

                last_s = [None]

                def s_mm(qb, kc):
                    qsl = slice(qb * QB, (qb + 1) * QB)
                    st = sps.tile([128, 2 * QB], F32, tag="s", name="s")
                    for h in (0, 1):
                        hsl = slice(h * D, (h + 1) * D)
                        mm = nc.tensor.matmul(
                            st[:, h * QB:(h + 1) * QB],
                            lhsT=kt[hsl, kc * KC:(kc + 1) * KC],
                            rhs=qt[hsl, qsl], start=True, stop=True)
                    last_s[0] = mm.ins
                    return st

                def do_exp(st):
                    p = ptp.tile([128, 2 * QB], BF16, tag="p", name="p")
                    nc.scalar.activation(p[:, 0:ACOL], st[:, 0:ACOL],
                                         AF.Exp, scale=0.125)
                    nc.vector.tensor_scalar(
                        out=p[:, ACOL:2 * QB].bitcast(I16),
                        in0=st[:, ACOL:2 * QB],
                        scalar1=EXP_MUL, scalar2=EXP_ADD,
                        op0=ALU.mult, op1=ALU.add)
                    return p

                pv_box = [None]     # current q-block's PV PSUM tile

                def pv_mm(p, pkc):
                    if pkc == 0:
                        # allocate here (in program order AFTER the previous
                        # epilogue's PSUM reads) so the pool WAR dep is right
                        pv_box[0] = pvp.tile([128, 2 * QB], F32, tag="pv",
                                             name="pv")
                    pv_ps = pv_box[0]
                    for h in (0, 1):
                        nc.tensor.matmul(
                            pv_ps[:, h * QB:(h + 1) * QB],
                            lhsT=vaug[:, h * VOFF + pkc * VSTR:
                                      h * VOFF + pkc * VSTR + 128],
                            rhs=p[:, h * QB:(h + 1) * QB],
                            start=(pkc == 0), stop=(pkc == NKC - 1))
                    return pv_ps

                def make_epilogue(qb, pv_ps):
                    dreg = msc.tile([1, 2 * QB], F32, tag="dreg", name="dreg")
                    rcp = msc.tile([1, 2 * QB], F32, tag="rcp", name="rcp")
                    bc = msc.tile([128, 2 * QB], F32, tag="bc", name="bc")
                    o2t = o2p.tile([HD, QB], BF16, tag="o2t", name="o2t")

                    def recip():
                        if debug and qb == 0:
                            pvc = msc.tile([128, 2 * QB], F32, tag="pvc", name="pvc")
                            nc.vector.tensor_copy(out=pvc[:], in_=pv_ps[:])
                            nc.sync.dma_start(out=dbg["pv0"][:], in_=pvc[:])
                        # recip_approx_fast mis-reads PSUM at partition
                        # base 64; stage the denom row to SBUF first
                        nc.vector.tensor_copy(out=dreg[:], in_=pv_ps[64:65, :])
                        nc.vector.reciprocal_approx_fast(
                            out=rcp[:], in_=dreg[:])
                        if debug and qb == 0:
                            nc.sync.dma_start(out=dbg["rcp0"][:], in_=rcp[:])

                    def bcast():
                        nc.gpsimd.partition_broadcast(bc[:], rcp[:])

                    def norm():
                        for h in (0, 1):
                            nc.vector.tensor_tensor(
                                out=o2t[h * D:(h + 1) * D, :],
                                in0=pv_ps[0:64, h * QB:(h + 1) * QB],
                                in1=bc[0:64, h * QB:(h + 1) * QB],
                                op=ALU.mult)
                        if debug and qb == 0:
                            nc.sync.dma_start(out=dbg["o2t0"][:], in_=o2t[:])
                        return o2t
                    return recip, bcast, norm

                def oproj_chunk(qb, o2t, qs, eng):
                    s_anchor = last_s[0]
                    po = pjp.tile([128, QB], F32, tag="pj", name="pj")
                    mm = nc.tensor.matmul(po[:, 0:C],
                                          lhsT=o2t[:, qs * 128:(qs + 1) * 128],
                                          rhs=wo_s[:], start=True, stop=True)
                    if s_anchor is not None:
                        # keep the scheduler from hoisting this ahead of the
                        # S stream
                        add_dep_helper(mm.ins, s_anchor, False,
                                       "outproj after S stream")
                    ob = obp.tile([128, C], F32, tag="ob", name="ob")
                    nc.scalar.copy(ob[:], po[:, 0:C])
                    r0 = qb * QB + qs * 128
                    eng.dma_start(out=out[r0:r0 + 128, :], in_=ob[:])

                # ---- main loop: PV runs one kc-step behind the exp ----
                pend_pv = []        # (p_tile, pkc) not yet emitted
                todo = {}           # kc slot -> [callables] (prev-qb epilogue)
                s_cur = s_mm(0, 0)
                for qb in range(NQB):
                    for kc in range(NKC):
                        if kc + 1 < NKC:
                            nxt = (qb, kc + 1)
                        elif qb + 1 < NQB:
                            nxt = (qb + 1, 0)
                        else:
                            nxt = None
                        s_next = s_mm(*nxt) if nxt else None
                        # emit pending PV pairs; a pair with pkc==0 must wait
                        # until the previous epilogue's PSUM reads (emitted at
                        # kc<=3) are in program order before it
                        budget = 2 if len(pend_pv) > 1 else 1
                        while pend_pv and budget > 0:
                            if qb > 0 and kc < 4 and pend_pv[0][1] == 0:
                                break
                            p_t, pkc = pend_pv.pop(0)
                            pv_mm(p_t, pkc)
                            budget -= 1
                        # previous q-block epilogue pieces
                        for fn in todo.pop(kc, ()):
                            fn()
                        # drip-feed remaining projections (qb=0 only)
                        n_items = 2 if (qb == 0 and kc < 8) else 1
                        for _ in range(n_items):
                            if pending_proj:
                                kind, idx = pending_proj.pop()
                                proj_v(idx) if kind == "v" else proj_qk(kind, idx)
                        if kc == 16 and qb < NQB - 1:
                            proj_qk("q", qb + 1)
                        if debug and qb == 1 and kc == 20:
                            nc.sync.dma_start(out=dbg["kt"][:], in_=kt[:])
                            nc.sync.dma_start(out=dbg["qt"][:], in_=qt[:])
                            nc.sync.dma_start(out=dbg["va"][:], in_=vaug[:])
                        # exp for this step; its PV queued for a later step
                        p = do_exp(s_cur)
                        if debug and qb == 0 and kc == 0:
                            nc.sync.dma_start(out=dbg["p00"][:], in_=p[:])
                        pend_pv.append((p, kc))
                        s_cur = s_next
                    if qb + 1 < NQB:
                        # schedule this q-block's epilogue into the next block
                        # (the PV tile is resolved at call time via pv_box —
                        # by kc=0 of qb+1 all 32 PV pairs of qb are... the
                        # last pair lands at kc=0; recip runs at kc=1)
                        pv_ps = pv_box[0]
                        recip, bcast, norm = make_epilogue(qb, pv_ps)
                        o2t_box = []
                        todo[1] = [recip]
                        todo[2] = [bcast]
                        todo[3] = [lambda n=norm, b=o2t_box: b.append(n())]
                        todo[10] = [lambda q=qb, b=o2t_box:
                                    oproj_chunk(q, b[0], 0, nc.sync),
                                    lambda q=qb, b=o2t_box:
                                    oproj_chunk(q, b[0], 1, nc.gpsimd)]
                        todo[13] = [lambda q=qb, b=o2t_box:
                                    oproj_chunk(q, b[0], 2, nc.sync),
                                    lambda q=qb, b=o2t_box:
                                    oproj_chunk(q, b[0], 3, nc.gpsimd)]
                    else:
                        # tail: drain the last PV pair, then the epilogue
                        while pend_pv:
                            p_t, pkc = pend_pv.pop(0)
                            pv_mm(p_t, pkc)
                        recip, bcast, norm = make_epilogue(qb, pv_box[0])
                        recip()
                        bcast()
                        o2t = norm()
                        for qs in range(4):
                            oproj_chunk(qb, o2t, qs,
                                        nc.sync if qs % 2 == 0 else nc.gpsimd)

    nc.compile()
    return nc


_NC_CACHE = {}


def _get_nc():
    if "nc" not in _NC_CACHE:
        _NC_CACHE["nc"] = build_nc()
    return _NC_CACHE["nc"]


def kernel(x, Wq, bq, Wk, bk, Wv, bv, Wo, bo):
    x = np.asarray(x, dtype=np.float32)
    bf = ml_dtypes.bfloat16
    nc = _get_nc()

    in_maps = []
    for c in range(8):
        b, p = c // 4, c % 4
        cs = slice(p * HD, (p + 1) * HD)
        in_maps.append({
            "xT": np.ascontiguousarray(x[b].T).astype(bf),
            "wq": np.ascontiguousarray(Wq[:, cs]).astype(bf),
            "wk": np.ascontiguousarray(Wk[:, cs]).astype(bf),
            "wv": np.ascontiguousarray(Wv[:, cs]).astype(bf),
            "wo": np.ascontiguousarray(Wo[cs, :]).astype(bf),
            "bq": np.asarray(bq[cs], np.float32).reshape(HD, 1).copy(),
            "bk": np.asarray(bk[cs], np.float32).reshape(HD, 1).copy(),
            "bv": np.asarray(bv[cs], np.float32).reshape(1, HD).astype(bf),
        })

    res = run_bass_kernel_spmd(nc, in_maps, core_ids=list(range(8)))

    out = np.zeros((2, N, C), np.float32)
    for c in range(8):
        out[c // 4] += res.results[c]["out"]
    out += np.asarray(bo, np.float32)[None, None, :]
    return out


# revision 6
# speedup vs baseline: 1.2519x; 1.2519x over previous
"""Multi-head attention (B=2, N=4096, C=512, H=8, D=64) on 8 TRN2 NeuronCores.

Sharding: core c handles batch b = c // 4 and head-pair p = c % 4
(heads 2p, 2p+1, i.e. channels [128p, 128p+128) of the QKV projections).
Each core computes a partial output projection O_loc @ Wo_loc; the host
sums the 4 partials per batch and adds bo. No collectives needed.

v2 over the 334us baseline (which was ScalarE-exp-bound at ~294us busy):
  - The exp is split between ScalarE (exact LUT exp on S cols [0:ACOL])
    and VectorE (cols [ACOL:1024] via a single tensor_scalar:
    y = s*23.083 + 16248.7 converted to int16, whose bit pattern IS
    bf16(exp(s/8)) by the Schraudolph trick; ~1.8% per-element RMS error
    that washes out to ~1e-3 end-to-end after the softmax ratio).
  - The PV matmuls run one kc-step behind the exp so the PE's in-order
    queue never waits on ACT/DVE within a step.
  - PSUM (8 banks): S double-buffer [128,1024]x2 (4) + single PV
    accumulator [128,1024] (2) + proj pool [128,512]x2 (2).  The PV
    pairs of each q-block's first steps are deferred until the previous
    block's epilogue TTs free the PV banks, then caught up 2/step.
  - Epilogue per q-block: reciprocal_approx_fast on the PSUM denominator
    row, one gpsimd partition_broadcast, two fused normalize-TTs reading
    PV PSUM directly (no o2tu staging), out-proj chunks with ScalarE
    PSUM->SBUF copies (ACT has slack), DMA per 128-row chunk.
"""
import numpy as np
import ml_dtypes

import concourse.bass as bass
import concourse.mybir as mybir
import concourse.tile as tile
from concourse.tile_rust import add_dep_helper
from concourse import bacc
from concourse.bass_utils import run_bass_kernel_spmd

F32 = mybir.dt.float32
BF16 = mybir.dt.bfloat16
I16 = mybir.dt.int16
AF = mybir.ActivationFunctionType
ALU = mybir.AluOpType

N = 4096
C = 512
HD = 128          # channels per core (2 heads x 64)
D = 64
QB = 512          # q-block
NQB = N // QB     # 8
KC = 128          # key chunk
NKC = N // KC     # 32
VSTR = 65         # [V(64) | ones] stride inside vaug
VOFF = 2144       # head offset inside the combined vaug tile

N_DVE = 116       # of the 256 kc-steps, this many exp on VectorE
# Schraudolph: bits_bf16(exp(s/8)) ~= round(s * (2^7*log2e/8) + (127*2^7 - C))
EXP_MUL = (2.0 ** 7) * 1.4426950408889634 / 8.0    # 23.0831206...
EXP_ADD = 127.0 * 128.0 - 7.3                      # 16248.7


def _dve_step(i):
    # Bresenham spread of N_DVE VectorE-exp steps over the 256 steps
    return (i * N_DVE) // 256 != ((i + 1) * N_DVE) // 256


def build_nc(debug=False):
    nc = bacc.Bacc(None, target_bir_lowering=False)

    xT = nc.declare_dram_parameter("xT", [C, N], BF16, isOutput=False)
    wq = nc.declare_dram_parameter("wq", [C, HD], BF16, isOutput=False)
    wk = nc.declare_dram_parameter("wk", [C, HD], BF16, isOutput=False)
    wv = nc.declare_dram_parameter("wv", [C, HD], BF16, isOutput=False)
    wo = nc.declare_dram_parameter("wo", [HD, C], BF16, isOutput=False)
    bq = nc.declare_dram_parameter("bq", [HD, 1], F32, isOutput=False)
    bk = nc.declare_dram_parameter("bk", [HD, 1], F32, isOutput=False)
    bv = nc.declare_dram_parameter("bv", [1, HD], BF16, isOutput=False)
    out = nc.declare_dram_parameter("out", [N, C], F32, isOutput=True)
    dbg = {}
    if debug:
        dbg["kt"] = nc.declare_dram_parameter("d_kt", [HD, N], BF16, isOutput=True)
        dbg["qt"] = nc.declare_dram_parameter("d_qt", [HD, N], BF16, isOutput=True)
        dbg["va"] = nc.declare_dram_parameter("d_va", [128, 2 * VOFF], BF16, isOutput=True)
        dbg["p00"] = nc.declare_dram_parameter("d_p00", [128, 2 * QB], BF16, isOutput=True)
        dbg["pv0"] = nc.declare_dram_parameter("d_pv0", [128, 2 * QB], F32, isOutput=True)
        dbg["rcp0"] = nc.declare_dram_parameter("d_rcp0", [1, 2 * QB], F32, isOutput=True)
        dbg["o2t0"] = nc.declare_dram_parameter("d_o2t0", [HD, QB], BF16, isOutput=True)

    with tile.TileContext(nc) as tc:
        with (
            tc.tile_pool(name="const", bufs=1) as cpool,
            tc.tile_pool(name="big", bufs=1) as bpool,
        ):
            # Constants / weights in SBUF
            xt = [cpool.tile([128, N], BF16, tag=f"xt{c}", name=f"xt{c}") for c in range(4)]
            wq_s = cpool.tile([128, C], BF16, tag="wq")
            wk_s = cpool.tile([128, C], BF16, tag="wk")
            wv_s = cpool.tile([128, C], BF16, tag="wv")
            wo_s = cpool.tile([HD, C], BF16, tag="wo")
            bq_s = cpool.tile([HD, 1], F32, tag="bq")
            bk_s = cpool.tile([HD, 1], F32, tag="bk")
            bv_s = cpool.tile([1, HD], BF16, tag="bv")
            ones_s = cpool.tile([1, 128], BF16, tag="ones")

            # Critical-path-first DMA order (per-DMA first-byte latency is
            # ~1us, so keep the prefix short): K/Q weights as single strided
            # DMAs, then xT block 0, then everything else. Two DGE queues.
            dma_engines = [nc.sync, nc.gpsimd]
            wk_r = wk[:].rearrange("(c p) m -> p c m", p=128)
            wq_r = wq[:].rearrange("(c p) m -> p c m", p=128)
            wv_r = wv[:].rearrange("(c p) m -> p c m", p=128)
            nc.sync.dma_start(
                out=wk_s[:].rearrange("p (c m) -> p c m", c=4), in_=wk_r)
            nc.gpsimd.dma_start(
                out=wq_s[:].rearrange("p (c m) -> p c m", c=4), in_=wq_r)
            for c in range(4):
                # tiny prefix: lets a 128-position K projection (and so the
                # first S matmul) start ~10us earlier
                eng = dma_engines[c % 2]
                eng.dma_start(out=xt[c][:, 0:128],
                              in_=xT[c * 128:(c + 1) * 128, 0:128])
            for blk in range(NQB):
                bsl = (slice(128, QB) if blk == 0
                       else slice(blk * QB, (blk + 1) * QB))
                for c in range(4):
                    eng = dma_engines[(blk * 4 + c) % 2]
                    eng.dma_start(out=xt[c][:, bsl],
                                  in_=xT[c * 128:(c + 1) * 128, bsl])
                if blk == 0:
                    nc.sync.dma_start(out=bk_s[:], in_=bk[:])
                    nc.gpsimd.dma_start(out=bq_s[:], in_=bq[:])
                    nc.sync.dma_start(out=wv_s[:].rearrange("p (c m) -> p c m", c=4), in_=wv_r)
                    nc.gpsimd.dma_start(out=bv_s[:], in_=bv[:])
            nc.sync.dma_start(out=wo_s[:], in_=wo[:])
            nc.vector.memset(ones_s[:], 1.0)

            # Persistent activations
            qt = bpool.tile([HD, N], BF16, tag="qt")
            kt = bpool.tile([HD, N], BF16, tag="kt")
            # combined V_aug for both heads: head h at cols [h*VOFF, ...),
            # per-kc [64 V | 1 ones] chunks at stride 65; padded so the PV
            # stationary can always load a 128-col window.
            vaug = bpool.tile([128, 2 * VOFF], BF16, tag="vaug")
            nc.vector.memset(vaug[:], 1.0)

            # warm the ACT exp table early so the ~2.7us load overlaps DMA
            wrm = bpool.tile([1, 128], BF16, tag="wrm")
            nc.scalar.activation(wrm[:], ones_s[:], AF.Exp)

            with (
                tc.tile_pool(name="sps", bufs=2, space="PSUM") as sps,
                tc.tile_pool(name="pvp", bufs=1, space="PSUM") as pvp,
                tc.tile_pool(name="pjp", bufs=2, space="PSUM") as pjp,
                tc.tile_pool(name="ptp", bufs=6) as ptp,
                tc.tile_pool(name="msc", bufs=2) as msc,
                tc.tile_pool(name="o2p", bufs=2) as o2p,
                tc.tile_pool(name="obp", bufs=4) as obp,
            ):
                def proj_qk(which, qb, lo=0, hi=QB):
                    sl = slice(qb * QB + lo, qb * QB + hi)
                    w_s, b_s, dst = ((wq_s, bq_s, qt) if which == "q"
                                     else (wk_s, bk_s, kt))
                    pq = pjp.tile([128, QB], F32, tag="pj", name="pj")
                    for c in range(4):
                        nc.tensor.matmul(pq[:, 0:hi - lo],
                                         lhsT=w_s[:, c * 128:(c + 1) * 128],
                                         rhs=xt[c][:, sl],
                                         start=(c == 0), stop=(c == 3))
                    nc.vector.tensor_scalar(out=dst[:, sl], in0=pq[:, 0:hi - lo],
                                            scalar1=b_s[:], scalar2=None,
                                            op0=ALU.add)

                def proj_v(pt):
                    psl = slice(pt * 128, (pt + 1) * 128)
                    pv = pjp.tile([128, QB], F32, tag="pj", name="pj")
                    for c in range(4):
                        nc.tensor.matmul(pv[:, 0:128], lhsT=xt[c][:, psl],
                                         rhs=wv_s[:, c * 128:(c + 1) * 128],
                                         start=(c == 0), stop=False)
                    nc.tensor.matmul(pv[:, 0:128], lhsT=ones_s[:], rhs=bv_s[:],
                                     start=False, stop=True)
                    # one copy for both heads: [128, 2, 64] strided into vaug
                    dst = vaug[:].rearrange("p (h z) -> p h z", h=2)
                    dst = dst[:, :, pt * VSTR:pt * VSTR + 64]
                    src = pv[:, 0:128].rearrange("p (h z) -> p h z", z=64)
                    nc.vector.tensor_copy(out=dst, in_=src)

                # minimal prologue; the rest of the projections interleave
                # into qb=0's kc loop, keeping PE, ACT and DVE busy
                proj_qk("k", 0, 0, 128)
                proj_qk("q", 0)
                proj_qk("k", 0, 128, QB)
                proj_v(0)
                proj_v(1)
                pending_proj = []
                for j in range(1, 8):
                    pending_proj.append(("v", j + 1))
                    pending_proj.append(("k", j))
                for pt in range(9, NKC):
                    pending_proj.append(("v", pt))
                pending_proj.reverse()  # pop() from the front

                last_s = [None]

                def s_mm(qb, kc):
                    qsl = slice(qb * QB, (qb + 1) * QB)
                    st = sps.tile([128, 2 * QB], F32, tag="s", name="s")
                    for h in (0, 1):
                        hsl = slice(h * D, (h + 1) * D)
                        mm = nc.tensor.matmul(
                            st[:, h * QB:(h + 1) * QB],
                            lhsT=kt[hsl, kc * KC:(kc + 1) * KC],
                            rhs=qt[hsl, qsl], start=True, stop=True)
                    last_s[0] = mm.ins
                    return st

                def do_exp(st, use_dve):
                    # whole step on ONE engine: a single p-tile writer, so
                    # ACT and DVE exp of adjacent steps run concurrently
                    # (tile-granular WAW tracking serialized a column split)
                    p = ptp.tile([128, 2 * QB], BF16, tag="p", name="p")
                    if use_dve:
                        nc.vector.tensor_scalar(
                            out=p[:].bitcast(I16), in0=st[:],
                            scalar1=EXP_MUL, scalar2=EXP_ADD,
                            op0=ALU.mult, op1=ALU.add)
                    else:
                        nc.scalar.activation(p[:], st[:], AF.Exp, scale=0.125)
                    return p

                pv_box = [None]     # current q-block's PV PSUM tile

                def pv_mm(p, pkc):
                    if pkc == 0:
                        # allocate here (in program order AFTER the previous
                        # epilogue's PSUM reads) so the pool WAR dep is right
                        pv_box[0] = pvp.tile([128, 2 * QB], F32, tag="pv",
                                             name="pv")
                    pv_ps = pv_box[0]
                    for h in (0, 1):
                        nc.tensor.matmul(
                            pv_ps[:, h * QB:(h + 1) * QB],
                            lhsT=vaug[:, h * VOFF + pkc * VSTR:
                                      h * VOFF + pkc * VSTR + 128],
                            rhs=p[:, h * QB:(h + 1) * QB],
                            start=(pkc == 0), stop=(pkc == NKC - 1))
                    return pv_ps

                def make_epilogue(qb, pv_ps):
                    dreg = msc.tile([1, 2 * QB], F32, tag="dreg", name="dreg")
                    rcp = msc.tile([1, 2 * QB], F32, tag="rcp", name="rcp")
                    bc = msc.tile([128, 2 * QB], F32, tag="bc", name="bc")
                    o2t = o2p.tile([HD, QB], BF16, tag="o2t", name="o2t")

                    def recip():
                        if debug and qb == 0:
                            pvc = msc.tile([128, 2 * QB], F32, tag="pvc", name="pvc")
                            nc.vector.tensor_copy(out=pvc[:], in_=pv_ps[:])
                            nc.sync.dma_start(out=dbg["pv0"][:], in_=pvc[:])
                        # recip_approx_fast mis-reads PSUM at partition
                        # base 64; stage the denom row to SBUF first
                        nc.scalar.copy(dreg[:], pv_ps[64:65, :])
                        nc.vector.reciprocal_approx_fast(
                            out=rcp[:], in_=dreg[:])
                        if debug and qb == 0:
                            nc.sync.dma_start(out=dbg["rcp0"][:], in_=rcp[:])

                    def bcast():
                        nc.gpsimd.partition_broadcast(bc[:], rcp[:])

                    def norm():
                        for h in (0, 1):
                            nc.vector.tensor_tensor(
                                out=o2t[h * D:(h + 1) * D, :],
                                in0=pv_ps[0:64, h * QB:(h + 1) * QB],
                                in1=bc[0:64, h * QB:(h + 1) * QB],
                                op=ALU.mult)
                        if debug and qb == 0:
                            nc.sync.dma_start(out=dbg["o2t0"][:], in_=o2t[:])
                        return o2t
                    return recip, bcast, norm

                def oproj_chunk(qb, o2t, qs, eng):
                    s_anchor = last_s[0]
                    po = pjp.tile([128, QB], F32, tag="pj", name="pj")
                    mm = nc.tensor.matmul(po[:, 0:C],
                                          lhsT=o2t[:, qs * 128:(qs + 1) * 128],
                                          rhs=wo_s[:], start=True, stop=True)
                    if s_anchor is not None:
                        # keep the scheduler from hoisting this ahead of the
                        # S stream
                        add_dep_helper(mm.ins, s_anchor, False,
                                       "outproj after S stream")
                    ob = obp.tile([128, C], F32, tag="ob", name="ob")
                    nc.scalar.copy(ob[:], po[:, 0:C])
                    r0 = qb * QB + qs * 128
                    eng.dma_start(out=out[r0:r0 + 128, :], in_=ob[:])

                # ---- main loop: PV runs one kc-step behind the exp ----
                pend_pv = []        # (p_tile, pkc) not yet emitted
                todo = {}           # kc slot -> [callables] (prev-qb epilogue)
                s_cur = s_mm(0, 0)
                for qb in range(NQB):
                    for kc in range(NKC):
                        if kc + 1 < NKC:
                            nxt = (qb, kc + 1)
                        elif qb + 1 < NQB:
                            nxt = (qb + 1, 0)
                        else:
                            nxt = None
                        s_next = s_mm(*nxt) if nxt else None
                        # emit pending PV pairs; a pair with pkc==0 must wait
                        # until the previous epilogue's PSUM reads (emitted at
                        # kc<=3) are in program order before it
                        budget = 2 if len(pend_pv) > 1 else 1
                        while pend_pv and budget > 0:
                            if qb > 0 and kc < 4 and pend_pv[0][1] == 0:
                                break
                            p_t, pkc = pend_pv.pop(0)
                            pv_mm(p_t, pkc)
                            budget -= 1
                        # previous q-block epilogue pieces
                        for fn in todo.pop(kc, ()):
                            fn()
                        # drip-feed remaining projections (qb=0 only)
                        n_items = 2 if (qb == 0 and kc < 8) else 1
                        for _ in range(n_items):
                            if pending_proj:
                                kind, idx = pending_proj.pop()
                                proj_v(idx) if kind == "v" else proj_qk(kind, idx)
                        if kc == 16 and qb < NQB - 1:
                            proj_qk("q", qb + 1)
                        if debug and qb == 1 and kc == 20:
                            nc.sync.dma_start(out=dbg["kt"][:], in_=kt[:])
                            nc.sync.dma_start(out=dbg["qt"][:], in_=qt[:])
                            nc.sync.dma_start(out=dbg["va"][:], in_=vaug[:])
                        # exp for this step; its PV queued for a later step
                        p = do_exp(s_cur, _dve_step(qb * NKC + kc))
                        if debug and qb == 0 and kc == 0:
                            nc.sync.dma_start(out=dbg["p00"][:], in_=p[:])
                        pend_pv.append((p, kc))
                        s_cur = s_next
                    if qb + 1 < NQB:
                        # schedule this q-block's epilogue into the next block
                        # (the PV tile is resolved at call time via pv_box —
                        # by kc=0 of qb+1 all 32 PV pairs of qb are... the
                        # last pair lands at kc=0; recip runs at kc=1)
                        pv_ps = pv_box[0]
                        recip, bcast, norm = make_epilogue(qb, pv_ps)
                        o2t_box = []
                        todo[1] = [recip]
                        todo[2] = [bcast]
                        todo[3] = [lambda n=norm, b=o2t_box: b.append(n())]
                        todo[10] = [lambda q=qb, b=o2t_box:
                                    oproj_chunk(q, b[0], 0, nc.sync),
                                    lambda q=qb, b=o2t_box:
                                    oproj_chunk(q, b[0], 1, nc.gpsimd)]
                        todo[13] = [lambda q=qb, b=o2t_box:
                                    oproj_chunk(q, b[0], 2, nc.sync),
                                    lambda q=qb, b=o2t_box:
                                    oproj_chunk(q, b[0], 3, nc.gpsimd)]
                    else:
                        # tail: drain the last PV pair, then the epilogue
                        while pend_pv:
                            p_t, pkc = pend_pv.pop(0)
                            pv_mm(p_t, pkc)
                        recip, bcast, norm = make_epilogue(qb, pv_box[0])
                        recip()
                        bcast()
                        o2t = norm()
                        for qs in range(4):
                            oproj_chunk(qb, o2t, qs,
                                        nc.sync if qs % 2 == 0 else nc.gpsimd)

    nc.compile()
    return nc


_NC_CACHE = {}


def _get_nc():
    if "nc" not in _NC_CACHE:
        _NC_CACHE["nc"] = build_nc()
    return _NC_CACHE["nc"]


def kernel(x, Wq, bq, Wk, bk, Wv, bv, Wo, bo):
    x = np.asarray(x, dtype=np.float32)
    bf = ml_dtypes.bfloat16
    nc = _get_nc()

    in_maps = []
    for c in range(8):
        b, p = c // 4, c % 4
        cs = slice(p * HD, (p + 1) * HD)
        in_maps.append({
            "xT": np.ascontiguousarray(x[b].T).astype(bf),
            "wq": np.ascontiguousarray(Wq[:, cs]).astype(bf),
            "wk": np.ascontiguousarray(Wk[:, cs]).astype(bf),
            "wv": np.ascontiguousarray(Wv[:, cs]).astype(bf),
            "wo": np.ascontiguousarray(Wo[cs, :]).astype(bf),
            "bq": np.asarray(bq[cs], np.float32).reshape(HD, 1).copy(),
            "bk": np.asarray(bk[cs], np.float32).reshape(HD, 1).copy(),
            "bv": np.asarray(bv[cs], np.float32).reshape(1, HD).astype(bf),
        })

    res = run_bass_kernel_spmd(nc, in_maps, core_ids=list(range(8)))

    out = np.zeros((2, N, C), np.float32)
    for c in range(8):
        out[c // 4] += res.results[c]["out"]
    out += np.asarray(bo, np.float32)[None, None, :]
    return out


# revision 7
# speedup vs baseline: 1.2700x; 1.0144x over previous
"""Multi-head attention (B=2, N=4096, C=512, H=8, D=64) on 8 TRN2 NeuronCores.

Sharding: core c handles batch b = c // 4 and head-pair p = c % 4
(heads 2p, 2p+1, i.e. channels [128p, 128p+128) of the QKV projections).
Each core computes a partial output projection O_loc @ Wo_loc; the host
sums the 4 partials per batch and adds bo. No collectives needed.

v2 over the 334us baseline (which was ScalarE-exp-bound at ~294us busy):
  - The exp is split between ScalarE (exact LUT exp on S cols [0:ACOL])
    and VectorE (cols [ACOL:1024] via a single tensor_scalar:
    y = s*23.083 + 16248.7 converted to int16, whose bit pattern IS
    bf16(exp(s/8)) by the Schraudolph trick; ~1.8% per-element RMS error
    that washes out to ~1e-3 end-to-end after the softmax ratio).
  - The PV matmuls run one kc-step behind the exp so the PE's in-order
    queue never waits on ACT/DVE within a step.
  - PSUM (8 banks): S double-buffer [128,1024]x2 (4) + single PV
    accumulator [128,1024] (2) + proj pool [128,512]x2 (2).  The PV
    pairs of each q-block's first steps are deferred until the previous
    block's epilogue TTs free the PV banks, then caught up 2/step.
  - Epilogue per q-block: reciprocal_approx_fast on the PSUM denominator
    row, one gpsimd partition_broadcast, two fused normalize-TTs reading
    PV PSUM directly (no o2tu staging), out-proj chunks with ScalarE
    PSUM->SBUF copies (ACT has slack), DMA per 128-row chunk.
"""
import numpy as np
import ml_dtypes

import concourse.bass as bass
import concourse.mybir as mybir
import concourse.tile as tile
from concourse.tile_rust import add_dep_helper
from concourse import bacc
from concourse.bass_utils import run_bass_kernel_spmd

F32 = mybir.dt.float32
BF16 = mybir.dt.bfloat16
I16 = mybir.dt.int16
AF = mybir.ActivationFunctionType
ALU = mybir.AluOpType

N = 4096
C = 512
HD = 128          # channels per core (2 heads x 64)
D = 64
QB = 512          # q-block
NQB = N // QB     # 8
KC = 128          # key chunk
NKC = N // KC     # 32
VSTR = 65         # [V(64) | ones] stride inside vaug
VOFF = 2144       # head offset inside the combined vaug tile

N_DVE = 116       # of the 256 kc-steps, this many exp on VectorE
# Schraudolph: bits_bf16(exp(s/8)) ~= round(s * (2^7*log2e/8) + (127*2^7 - C))
EXP_MUL = (2.0 ** 7) * 1.4426950408889634 / 8.0    # 23.0831206...
EXP_ADD = 127.0 * 128.0 - 7.3                      # 16248.7


def _dve_step(i):
    # Bresenham spread of N_DVE VectorE-exp steps over the 256 steps
    return (i * N_DVE) // 256 != ((i + 1) * N_DVE) // 256


def build_nc(debug=False):
    nc = bacc.Bacc(None, target_bir_lowering=False)

    xT = nc.declare_dram_parameter("xT", [C, N], BF16, isOutput=False)
    wq = nc.declare_dram_parameter("wq", [C, HD], BF16, isOutput=False)
    wk = nc.declare_dram_parameter("wk", [C, HD], BF16, isOutput=False)
    wv = nc.declare_dram_parameter("wv", [C, HD], BF16, isOutput=False)
    wo = nc.declare_dram_parameter("wo", [HD, C], BF16, isOutput=False)
    bq = nc.declare_dram_parameter("bq", [HD, 1], F32, isOutput=False)
    bk = nc.declare_dram_parameter("bk", [HD, 1], F32, isOutput=False)
    bv = nc.declare_dram_parameter("bv", [1, HD], BF16, isOutput=False)
    out = nc.declare_dram_parameter("out", [N, C], F32, isOutput=True)
    dbg = {}
    if debug:
        dbg["kt"] = nc.declare_dram_parameter("d_kt", [HD, N], BF16, isOutput=True)
        dbg["qt"] = nc.declare_dram_parameter("d_qt", [HD, N], BF16, isOutput=True)
        dbg["va"] = nc.declare_dram_parameter("d_va", [128, 2 * VOFF], BF16, isOutput=True)
        dbg["p00"] = nc.declare_dram_parameter("d_p00", [128, 2 * QB], BF16, isOutput=True)
        dbg["pv0"] = nc.declare_dram_parameter("d_pv0", [128, 2 * QB], F32, isOutput=True)
        dbg["rcp0"] = nc.declare_dram_parameter("d_rcp0", [1, 2 * QB], F32, isOutput=True)
        dbg["o2t0"] = nc.declare_dram_parameter("d_o2t0", [HD, QB], BF16, isOutput=True)

    with tile.TileContext(nc) as tc:
        with (
            tc.tile_pool(name="const", bufs=1) as cpool,
            tc.tile_pool(name="big", bufs=1) as bpool,
        ):
            # Constants / weights in SBUF
            xt = [cpool.tile([128, N], BF16, tag=f"xt{c}", name=f"xt{c}") for c in range(4)]
            wq_s = cpool.tile([128, C], BF16, tag="wq")
            wk_s = cpool.tile([128, C], BF16, tag="wk")
            wv_s = cpool.tile([128, C], BF16, tag="wv")
            wo_s = cpool.tile([HD, C], BF16, tag="wo")
            bq_s = cpool.tile([HD, 1], F32, tag="bq")
            bk_s = cpool.tile([HD, 1], F32, tag="bk")
            bv_s = cpool.tile([1, HD], BF16, tag="bv")
            ones_s = cpool.tile([1, 128], BF16, tag="ones")

            # Critical-path-first DMA order (per-DMA first-byte latency is
            # ~1us, so keep the prefix short): K/Q weights as single strided
            # DMAs, then xT block 0, then everything else. Two DGE queues.
            dma_engines = [nc.sync, nc.gpsimd]
            wk_r = wk[:].rearrange("(c p) m -> p c m", p=128)
            wq_r = wq[:].rearrange("(c p) m -> p c m", p=128)
            wv_r = wv[:].rearrange("(c p) m -> p c m", p=128)
            nc.sync.dma_start(
                out=wk_s[:].rearrange("p (c m) -> p c m", c=4), in_=wk_r)
            nc.gpsimd.dma_start(
                out=wq_s[:].rearrange("p (c m) -> p c m", c=4), in_=wq_r)
            for c in range(4):
                # tiny prefix: lets a 128-position K projection (and so the
                # first S matmul) start ~10us earlier
                eng = dma_engines[c % 2]
                eng.dma_start(out=xt[c][:, 0:128],
                              in_=xT[c * 128:(c + 1) * 128, 0:128])
            for blk in range(NQB):
                bsl = (slice(128, QB) if blk == 0
                       else slice(blk * QB, (blk + 1) * QB))
                for c in range(4):
                    eng = dma_engines[(blk * 4 + c) % 2]
                    eng.dma_start(out=xt[c][:, bsl],
                                  in_=xT[c * 128:(c + 1) * 128, bsl])
                if blk == 0:
                    nc.sync.dma_start(out=bk_s[:], in_=bk[:])
                    nc.gpsimd.dma_start(out=bq_s[:], in_=bq[:])
                    nc.sync.dma_start(out=wv_s[:].rearrange("p (c m) -> p c m", c=4), in_=wv_r)
                    nc.gpsimd.dma_start(out=bv_s[:], in_=bv[:])
            nc.sync.dma_start(out=wo_s[:], in_=wo[:])
            nc.vector.memset(ones_s[:], 1.0)

            # Persistent activations
            qt = bpool.tile([HD, N], BF16, tag="qt")
            kt = bpool.tile([HD, N], BF16, tag="kt")
            # combined V_aug for both heads: head h at cols [h*VOFF, ...),
            # per-kc [64 V | 1 ones] chunks at stride 65; padded so the PV
            # stationary can always load a 128-col window.
            vaug = bpool.tile([128, 2 * VOFF], BF16, tag="vaug")
            nc.vector.memset(vaug[:], 1.0)

            # warm the ACT exp table early so the ~2.7us load overlaps DMA
            wrm = bpool.tile([1, 128], BF16, tag="wrm")
            nc.scalar.activation(wrm[:], ones_s[:], AF.Exp)

            with (
                tc.tile_pool(name="sps", bufs=3, space="PSUM") as sps,
                tc.tile_pool(name="pvp", bufs=1, space="PSUM") as pvp,
                tc.tile_pool(name="ptp", bufs=6) as ptp,
                tc.tile_pool(name="msc", bufs=2) as msc,
                tc.tile_pool(name="o2p", bufs=2) as o2p,
                tc.tile_pool(name="obp", bufs=4) as obp,
            ):
                def proj_qk(which, qb, lo=0, hi=QB):
                    sl = slice(qb * QB + lo, qb * QB + hi)
                    w_s, b_s, dst = ((wq_s, bq_s, qt) if which == "q"
                                     else (wk_s, bk_s, kt))
                    pq = sps.tile([128, 2 * QB], F32, tag="s", name="s")
                    for c in range(4):
                        nc.tensor.matmul(pq[:, 0:hi - lo],
                                         lhsT=w_s[:, c * 128:(c + 1) * 128],
                                         rhs=xt[c][:, sl],
                                         start=(c == 0), stop=(c == 3))
                    nc.vector.tensor_scalar(out=dst[:, sl], in0=pq[:, 0:hi - lo],
                                            scalar1=b_s[:], scalar2=None,
                                            op0=ALU.add)

                def proj_v(pt):
                    psl = slice(pt * 128, (pt + 1) * 128)
                    pv = sps.tile([128, 2 * QB], F32, tag="s", name="s")
                    for c in range(4):
                        nc.tensor.matmul(pv[:, 0:128], lhsT=xt[c][:, psl],
                                         rhs=wv_s[:, c * 128:(c + 1) * 128],
                                         start=(c == 0), stop=False)
                    nc.tensor.matmul(pv[:, 0:128], lhsT=ones_s[:], rhs=bv_s[:],
                                     start=False, stop=True)
                    # one copy for both heads: [128, 2, 64] strided into vaug
                    dst = vaug[:].rearrange("p (h z) -> p h z", h=2)
                    dst = dst[:, :, pt * VSTR:pt * VSTR + 64]
                    src = pv[:, 0:128].rearrange("p (h z) -> p h z", z=64)
                    nc.vector.tensor_copy(out=dst, in_=src)

                # minimal prologue; the rest of the projections interleave
                # into qb=0's kc loop, keeping PE, ACT and DVE busy
                proj_qk("k", 0, 0, 128)
                proj_qk("q", 0)
                proj_qk("k", 0, 128, QB)
                proj_v(0)
                proj_v(1)
                pending_proj = []
                for j in range(1, 8):
                    pending_proj.append(("v", j + 1))
                    pending_proj.append(("k", j))
                for pt in range(9, NKC):
                    pending_proj.append(("v", pt))
                pending_proj.reverse()  # pop() from the front

                last_s = [None]

                def s_mm(qb, kc):
                    qsl = slice(qb * QB, (qb + 1) * QB)
                    st = sps.tile([128, 2 * QB], F32, tag="s", name="s")
                    for h in (0, 1):
                        hsl = slice(h * D, (h + 1) * D)
                        mm = nc.tensor.matmul(
                            st[:, h * QB:(h + 1) * QB],
                            lhsT=kt[hsl, kc * KC:(kc + 1) * KC],
                            rhs=qt[hsl, qsl], start=True, stop=True)
                    last_s[0] = mm.ins
                    return st

                def do_exp(st, use_dve):
                    # whole step on ONE engine: a single p-tile writer, so
                    # ACT and DVE exp of adjacent steps run concurrently
                    # (tile-granular WAW tracking serialized a column split)
                    p = ptp.tile([128, 2 * QB], BF16, tag="p", name="p")
                    if use_dve:
                        nc.vector.tensor_scalar(
                            out=p[:].bitcast(I16), in0=st[:],
                            scalar1=EXP_MUL, scalar2=EXP_ADD,
                            op0=ALU.mult, op1=ALU.add)
                    else:
                        nc.scalar.activation(p[:], st[:], AF.Exp, scale=0.125)
                    return p

                pv_box = [None]     # current q-block's PV PSUM tile

                def pv_mm(p, pkc):
                    if pkc == 0:
                        # allocate here (in program order AFTER the previous
                        # epilogue's PSUM reads) so the pool WAR dep is right
                        pv_box[0] = pvp.tile([128, 2 * QB], F32, tag="pv",
                                             name="pv")
                    pv_ps = pv_box[0]
                    for h in (0, 1):
                        nc.tensor.matmul(
                            pv_ps[:, h * QB:(h + 1) * QB],
                            lhsT=vaug[:, h * VOFF + pkc * VSTR:
                                      h * VOFF + pkc * VSTR + 128],
                            rhs=p[:, h * QB:(h + 1) * QB],
                            start=(pkc == 0), stop=(pkc == NKC - 1))
                    return pv_ps

                def make_epilogue(qb, pv_ps):
                    dreg = msc.tile([1, 2 * QB], F32, tag="dreg", name="dreg")
                    rcp = msc.tile([1, 2 * QB], F32, tag="rcp", name="rcp")
                    bc = msc.tile([128, 2 * QB], F32, tag="bc", name="bc")
                    o2t = o2p.tile([HD, QB], BF16, tag="o2t", name="o2t")

                    def recip():
                        if debug and qb == 0:
                            pvc = msc.tile([128, 2 * QB], F32, tag="pvc", name="pvc")
                            nc.vector.tensor_copy(out=pvc[:], in_=pv_ps[:])
                            nc.sync.dma_start(out=dbg["pv0"][:], in_=pvc[:])
                        # recip_approx_fast mis-reads PSUM at partition
                        # base 64; stage the denom row to SBUF first
                        nc.scalar.copy(dreg[:], pv_ps[64:65, :])
                        nc.vector.reciprocal_approx_fast(
                            out=rcp[:], in_=dreg[:])
                        if debug and qb == 0:
                            nc.sync.dma_start(out=dbg["rcp0"][:], in_=rcp[:])

                    def bcast():
                        nc.gpsimd.partition_broadcast(bc[:], rcp[:])

                    def norm():
                        for h in (0, 1):
                            nc.vector.tensor_tensor(
                                out=o2t[h * D:(h + 1) * D, :],
                                in0=pv_ps[0:64, h * QB:(h + 1) * QB],
                                in1=bc[0:64, h * QB:(h + 1) * QB],
                                op=ALU.mult)
                        if debug and qb == 0:
                            nc.sync.dma_start(out=dbg["o2t0"][:], in_=o2t[:])
                        return o2t
                    return recip, bcast, norm

                def oproj_round(qb, o2t, rnd, eng):
                    s_anchor = last_s[0]
                    po = sps.tile([128, 2 * QB], F32, tag="s", name="s")
                    for i in (0, 1):
                        qs = rnd * 2 + i
                        mm = nc.tensor.matmul(
                            po[:, i * C:(i + 1) * C],
                            lhsT=o2t[:, qs * 128:(qs + 1) * 128],
                            rhs=wo_s[:], start=True, stop=True)
                        if s_anchor is not None:
                            # keep the scheduler from hoisting this ahead of
                            # the S stream
                            add_dep_helper(mm.ins, s_anchor, False,
                                           "outproj after S stream")
                    ob = obp.tile([128, 2 * C], F32, tag="ob", name="ob")
                    nc.scalar.copy(ob[:], po[:])
                    r0 = qb * QB + rnd * 256
                    dst = out[r0:r0 + 256, :].rearrange("(u p) c -> p u c", u=2)
                    eng.dma_start(out=dst,
                                  in_=ob[:].rearrange("p (u c) -> p u c", u=2))

                # ---- main loop: PV runs one kc-step behind the exp ----
                pend_pv = []        # (p_tile, pkc) not yet emitted
                todo = {}           # kc slot -> [callables] (prev-qb epilogue)
                s_cur = s_mm(0, 0)
                for qb in range(NQB):
                    for kc in range(NKC):
                        if kc + 1 < NKC:
                            nxt = (qb, kc + 1)
                        elif qb + 1 < NQB:
                            nxt = (qb + 1, 0)
                        else:
                            nxt = None
                        s_next = s_mm(*nxt) if nxt else None
                        # emit pending PV pairs; a pair with pkc==0 must wait
                        # until the previous epilogue's PSUM reads (emitted at
                        # kc<=3) are in program order before it
                        budget = 2 if len(pend_pv) > 1 else 1
                        while pend_pv and budget > 0:
                            if qb > 0 and kc < 4 and pend_pv[0][1] == 0:
                                break
                            p_t, pkc = pend_pv.pop(0)
                            pv_mm(p_t, pkc)
                            budget -= 1
                        # previous q-block epilogue pieces
                        for fn in todo.pop(kc, ()):
                            fn()
                        # drip-feed remaining projections (qb=0 only)
                        n_items = 2 if (qb == 0 and kc < 8) else 1
                        for _ in range(n_items):
                            if pending_proj:
                                kind, idx = pending_proj.pop()
                                proj_v(idx) if kind == "v" else proj_qk(kind, idx)
                        if kc == 16 and qb < NQB - 1:
                            proj_qk("q", qb + 1)
                        if debug and qb == 1 and kc == 20:
                            nc.sync.dma_start(out=dbg["kt"][:], in_=kt[:])
                            nc.sync.dma_start(out=dbg["qt"][:], in_=qt[:])
                            nc.sync.dma_start(out=dbg["va"][:], in_=vaug[:])
                        # exp for this step; its PV queued for a later step
                        p = do_exp(s_cur, _dve_step(qb * NKC + kc))
                        if debug and qb == 0 and kc == 0:
                            nc.sync.dma_start(out=dbg["p00"][:], in_=p[:])
                        pend_pv.append((p, kc))
                        s_cur = s_next
                    if qb + 1 < NQB:
                        # schedule this q-block's epilogue into the next block
                        # (the PV tile is resolved at call time via pv_box —
                        # by kc=0 of qb+1 all 32 PV pairs of qb are... the
                        # last pair lands at kc=0; recip runs at kc=1)
                        pv_ps = pv_box[0]
                        recip, bcast, norm = make_epilogue(qb, pv_ps)
                        o2t_box = []
                        todo[1] = [recip]
                        todo[2] = [bcast]
                        todo[3] = [lambda n=norm, b=o2t_box: b.append(n())]
                        todo[10] = [lambda q=qb, b=o2t_box:
                                    oproj_round(q, b[0], 0, nc.sync)]
                        todo[13] = [lambda q=qb, b=o2t_box:
                                    oproj_round(q, b[0], 1, nc.gpsimd)]
                    else:
                        # tail: drain the last PV pair, then the epilogue
                        while pend_pv:
                            p_t, pkc = pend_pv.pop(0)
                            pv_mm(p_t, pkc)
                        recip, bcast, norm = make_epilogue(qb, pv_box[0])
                        recip()
                        bcast()
                        o2t = norm()
                        oproj_round(qb, o2t, 0, nc.sync)
                        oproj_round(qb, o2t, 1, nc.gpsimd)

    nc.compile()
    return nc


_NC_CACHE = {}


def _get_nc():
    if "nc" not in _NC_CACHE:
        _NC_CACHE["nc"] = build_nc()
    return _NC_CACHE["nc"]


def kernel(x, Wq, bq, Wk, bk, Wv, bv, Wo, bo):
    x = np.asarray(x, dtype=np.float32)
    bf = ml_dtypes.bfloat16
    nc = _get_nc()

    in_maps = []
    for c in range(8):
        b, p = c // 4, c % 4
        cs = slice(p * HD, (p + 1) * HD)
        in_maps.append({
            "xT": np.ascontiguousarray(x[b].T).astype(bf),
            "wq": np.ascontiguousarray(Wq[:, cs]).astype(bf),
            "wk": np.ascontiguousarray(Wk[:, cs]).astype(bf),
            "wv": np.ascontiguousarray(Wv[:, cs]).astype(bf),
            "wo": np.ascontiguousarray(Wo[cs, :]).astype(bf),
            "bq": np.asarray(bq[cs], np.float32).reshape(HD, 1).copy(),
            "bk": np.asarray(bk[cs], np.float32).reshape(HD, 1).copy(),
            "bv": np.asarray(bv[cs], np.float32).reshape(1, HD).astype(bf),
        })

    res = run_bass_kernel_spmd(nc, in_maps, core_ids=list(range(8)))

    out = np.zeros((2, N, C), np.float32)
    for c in range(8):
        out[c // 4] += res.results[c]["out"]
    out += np.asarray(bo, np.float32)[None, None, :]
    return out


# revision 8
# speedup vs baseline: 1.2793x; 1.0074x over previous
"""Multi-head attention (B=2, N=4096, C=512, H=8, D=64) on 8 TRN2 NeuronCores.

Sharding: core c handles batch b = c // 4 and head-pair p = c % 4
(heads 2p, 2p+1, i.e. channels [128p, 128p+128) of the QKV projections).
Each core computes a partial output projection O_loc @ Wo_loc; the host
sums the 4 partials per batch and adds bo. No collectives needed.

v2 over the 334us baseline (which was ScalarE-exp-bound at ~294us busy):
  - The exp is split between ScalarE (exact LUT exp on S cols [0:ACOL])
    and VectorE (cols [ACOL:1024] via a single tensor_scalar:
    y = s*23.083 + 16248.7 converted to int16, whose bit pattern IS
    bf16(exp(s/8)) by the Schraudolph trick; ~1.8% per-element RMS error
    that washes out to ~1e-3 end-to-end after the softmax ratio).
  - The PV matmuls run one kc-step behind the exp so the PE's in-order
    queue never waits on ACT/DVE within a step.
  - PSUM (8 banks): S double-buffer [128,1024]x2 (4) + single PV
    accumulator [128,1024] (2) + proj pool [128,512]x2 (2).  The PV
    pairs of each q-block's first steps are deferred until the previous
    block's epilogue TTs free the PV banks, then caught up 2/step.
  - Epilogue per q-block: reciprocal_approx_fast on the PSUM denominator
    row, one gpsimd partition_broadcast, two fused normalize-TTs reading
    PV PSUM directly (no o2tu staging), out-proj chunks with ScalarE
    PSUM->SBUF copies (ACT has slack), DMA per 128-row chunk.
"""
import numpy as np
import ml_dtypes

import concourse.bass as bass
import concourse.mybir as mybir
import concourse.tile as tile
from concourse.tile_rust import add_dep_helper
from concourse import bacc
from concourse.bass_utils import run_bass_kernel_spmd

F32 = mybir.dt.float32
BF16 = mybir.dt.bfloat16
I16 = mybir.dt.int16
AF = mybir.ActivationFunctionType
ALU = mybir.AluOpType

N = 4096
C = 512
HD = 128          # channels per core (2 heads x 64)
D = 64
QB = 512          # q-block
NQB = N // QB     # 8
KC = 128          # key chunk
NKC = N // KC     # 32
VSTR = 65         # [V(64) | ones] stride inside vaug
VOFF = 2144       # head offset inside the combined vaug tile

N_DVE = 116       # of the 256 kc-steps, this many exp on VectorE
# Schraudolph: bits_bf16(exp(s/8)) ~= round(s * (2^7*log2e/8) + (127*2^7 - C))
EXP_MUL = (2.0 ** 7) * 1.4426950408889634 / 8.0    # 23.0831206...
EXP_ADD = 127.0 * 128.0 - 7.3                      # 16248.7


def _dve_step(i):
    # Bresenham spread of N_DVE VectorE-exp steps over the 256 steps
    return (i * N_DVE) // 256 != ((i + 1) * N_DVE) // 256


def build_nc(debug=False):
    nc = bacc.Bacc(None, target_bir_lowering=False)

    xT = nc.declare_dram_parameter("xT", [C, N], BF16, isOutput=False)
    wq = nc.declare_dram_parameter("wq", [C, HD], BF16, isOutput=False)
    wk = nc.declare_dram_parameter("wk", [C, HD], BF16, isOutput=False)
    wv = nc.declare_dram_parameter("wv", [C, HD], BF16, isOutput=False)
    wo = nc.declare_dram_parameter("wo", [HD, C], BF16, isOutput=False)
    bq = nc.declare_dram_parameter("bq", [HD, 1], F32, isOutput=False)
    bk = nc.declare_dram_parameter("bk", [HD, 1], F32, isOutput=False)
    bv = nc.declare_dram_parameter("bv", [1, HD], BF16, isOutput=False)
    out = nc.declare_dram_parameter("out", [N, C], F32, isOutput=True)
    dbg = {}
    if debug:
        dbg["kt"] = nc.declare_dram_parameter("d_kt", [HD, N], BF16, isOutput=True)
        dbg["qt"] = nc.declare_dram_parameter("d_qt", [HD, N], BF16, isOutput=True)
        dbg["va"] = nc.declare_dram_parameter("d_va", [128, 2 * VOFF], BF16, isOutput=True)
        dbg["p00"] = nc.declare_dram_parameter("d_p00", [128, 2 * QB], BF16, isOutput=True)
        dbg["pv0"] = nc.declare_dram_parameter("d_pv0", [128, 2 * QB], F32, isOutput=True)
        dbg["rcp0"] = nc.declare_dram_parameter("d_rcp0", [1, 2 * QB], F32, isOutput=True)
        dbg["o2t0"] = nc.declare_dram_parameter("d_o2t0", [HD, QB], BF16, isOutput=True)

    with tile.TileContext(nc) as tc:
        with (
            tc.tile_pool(name="const", bufs=1) as cpool,
            tc.tile_pool(name="big", bufs=1) as bpool,
        ):
            # Constants / weights in SBUF
            xt = [cpool.tile([128, N], BF16, tag=f"xt{c}", name=f"xt{c}") for c in range(4)]
            wq_s = cpool.tile([128, C], BF16, tag="wq")
            wk_s = cpool.tile([128, C], BF16, tag="wk")
            wv_s = cpool.tile([128, C], BF16, tag="wv")
            wo_s = cpool.tile([HD, C], BF16, tag="wo")
            bq_s = cpool.tile([HD, 1], F32, tag="bq")
            bk_s = cpool.tile([HD, 1], F32, tag="bk")
            bv_s = cpool.tile([1, HD], BF16, tag="bv")
            ones_s = cpool.tile([1, 128], BF16, tag="ones")

            # Critical-path-first DMA order (per-DMA first-byte latency is
            # ~1us, so keep the prefix short): K/Q weights as single strided
            # DMAs, then xT block 0, then everything else. Two DGE queues.
            dma_engines = [nc.sync, nc.gpsimd]
            wk_r = wk[:].rearrange("(c p) m -> p c m", p=128)
            wq_r = wq[:].rearrange("(c p) m -> p c m", p=128)
            wv_r = wv[:].rearrange("(c p) m -> p c m", p=128)
            nc.sync.dma_start(
                out=wk_s[:].rearrange("p (c m) -> p c m", c=4), in_=wk_r)
            nc.gpsimd.dma_start(
                out=wq_s[:].rearrange("p (c m) -> p c m", c=4), in_=wq_r)
            for c in range(4):
                # tiny prefix: lets a 128-position K projection (and so the
                # first S matmul) start ~10us earlier
                eng = dma_engines[c % 2]
                eng.dma_start(out=xt[c][:, 0:128],
                              in_=xT[c * 128:(c + 1) * 128, 0:128])
            for blk in range(NQB):
                bsl = (slice(128, QB) if blk == 0
                       else slice(blk * QB, (blk + 1) * QB))
                for c in range(4):
                    eng = dma_engines[(blk * 4 + c) % 2]
                    eng.dma_start(out=xt[c][:, bsl],
                                  in_=xT[c * 128:(c + 1) * 128, bsl])
                if blk == 0:
                    nc.sync.dma_start(out=bk_s[:], in_=bk[:])
                    nc.gpsimd.dma_start(out=bq_s[:], in_=bq[:])
                    nc.sync.dma_start(out=wv_s[:].rearrange("p (c m) -> p c m", c=4), in_=wv_r)
                    nc.gpsimd.dma_start(out=bv_s[:], in_=bv[:])
            nc.sync.dma_start(out=wo_s[:], in_=wo[:])
            nc.vector.memset(ones_s[:], 1.0)

            # Persistent activations
            qt = bpool.tile([HD, N], BF16, tag="qt")
            kt = bpool.tile([HD, N], BF16, tag="kt")
            # combined V_aug for both heads: head h at cols [h*VOFF, ...),
            # per-kc [64 V | 1 ones] chunks at stride 65; padded so the PV
            # stationary can always load a 128-col window.
            vaug = bpool.tile([128, 2 * VOFF], BF16, tag="vaug")
            nc.vector.memset(vaug[:], 1.0)

            # warm the ACT exp table early so the ~2.7us load overlaps DMA
            wrm = bpool.tile([1, 128], BF16, tag="wrm")
            nc.scalar.activation(wrm[:], ones_s[:], AF.Exp)

            with (
                tc.tile_pool(name="sps", bufs=3, space="PSUM") as sps,
                tc.tile_pool(name="pvp", bufs=1, space="PSUM") as pvp,
                tc.tile_pool(name="ptp", bufs=10) as ptp,
                tc.tile_pool(name="msc", bufs=2) as msc,
                tc.tile_pool(name="o2p", bufs=2) as o2p,
                tc.tile_pool(name="obp", bufs=4) as obp,
            ):
                def proj_qk(which, qb, lo=0, hi=QB):
                    sl = slice(qb * QB + lo, qb * QB + hi)
                    w_s, b_s, dst = ((wq_s, bq_s, qt) if which == "q"
                                     else (wk_s, bk_s, kt))
                    pq = sps.tile([128, 2 * QB], F32, tag="s", name="s")
                    for c in range(4):
                        nc.tensor.matmul(pq[:, 0:hi - lo],
                                         lhsT=w_s[:, c * 128:(c + 1) * 128],
                                         rhs=xt[c][:, sl],
                                         start=(c == 0), stop=(c == 3))
                    nc.vector.tensor_scalar(out=dst[:, sl], in0=pq[:, 0:hi - lo],
                                            scalar1=b_s[:], scalar2=None,
                                            op0=ALU.add)

                def proj_v(pt):
                    psl = slice(pt * 128, (pt + 1) * 128)
                    pv = sps.tile([128, 2 * QB], F32, tag="s", name="s")
                    for c in range(4):
                        nc.tensor.matmul(pv[:, 0:128], lhsT=xt[c][:, psl],
                                         rhs=wv_s[:, c * 128:(c + 1) * 128],
                                         start=(c == 0), stop=False)
                    nc.tensor.matmul(pv[:, 0:128], lhsT=ones_s[:], rhs=bv_s[:],
                                     start=False, stop=True)
                    # one copy for both heads: [128, 2, 64] strided into vaug
                    dst = vaug[:].rearrange("p (h z) -> p h z", h=2)
                    dst = dst[:, :, pt * VSTR:pt * VSTR + 64]
                    src = pv[:, 0:128].rearrange("p (h z) -> p h z", z=64)
                    nc.vector.tensor_copy(out=dst, in_=src)

                # minimal prologue; the rest of the projections interleave
                # into qb=0's kc loop, keeping PE, ACT and DVE busy
                proj_qk("k", 0, 0, 128)
                proj_qk("q", 0)
                proj_qk("k", 0, 128, QB)
                proj_v(0)
                proj_v(1)
                pending_proj = []
                for j in range(1, 8):
                    pending_proj.append(("v", j + 1))
                    pending_proj.append(("k", j))
                for pt in range(9, NKC):
                    pending_proj.append(("v", pt))
                pending_proj.reverse()  # pop() from the front

                last_s = [None]

                def s_mm(qb, kc):
                    qsl = slice(qb * QB, (qb + 1) * QB)
                    st = sps.tile([128, 2 * QB], F32, tag="s", name="s")
                    for h in (0, 1):
                        hsl = slice(h * D, (h + 1) * D)
                        mm = nc.tensor.matmul(
                            st[:, h * QB:(h + 1) * QB],
                            lhsT=kt[hsl, kc * KC:(kc + 1) * KC],
                            rhs=qt[hsl, qsl], start=True, stop=True)
                    last_s[0] = mm.ins
                    return st

                def do_exp(st, use_dve):
                    # whole step on ONE engine: a single p-tile writer, so
                    # ACT and DVE exp of adjacent steps run concurrently
                    # (tile-granular WAW tracking serialized a column split)
                    p = ptp.tile([128, 2 * QB], BF16, tag="p", name="p")
                    if use_dve:
                        nc.vector.tensor_scalar(
                            out=p[:].bitcast(I16), in0=st[:],
                            scalar1=EXP_MUL, scalar2=EXP_ADD,
                            op0=ALU.mult, op1=ALU.add)
                    else:
                        nc.scalar.activation(p[:], st[:], AF.Exp, scale=0.125)
                    return p

                pv_box = [None]     # current q-block's PV PSUM tile

                def pv_mm(p, pkc):
                    if pkc == 0:
                        # allocate here (in program order AFTER the previous
                        # epilogue's PSUM reads) so the pool WAR dep is right
                        pv_box[0] = pvp.tile([128, 2 * QB], F32, tag="pv",
                                             name="pv")
                    pv_ps = pv_box[0]
                    for h in (0, 1):
                        nc.tensor.matmul(
                            pv_ps[:, h * QB:(h + 1) * QB],
                            lhsT=vaug[:, h * VOFF + pkc * VSTR:
                                      h * VOFF + pkc * VSTR + 128],
                            rhs=p[:, h * QB:(h + 1) * QB],
                            start=(pkc == 0), stop=(pkc == NKC - 1))
                    return pv_ps

                def make_epilogue(qb, pv_ps):
                    dreg = msc.tile([1, 2 * QB], F32, tag="dreg", name="dreg")
                    rcp = msc.tile([1, 2 * QB], F32, tag="rcp", name="rcp")
                    bc = msc.tile([128, 2 * QB], F32, tag="bc", name="bc")
                    o2t = o2p.tile([HD, QB], BF16, tag="o2t", name="o2t")

                    def recip():
                        if debug and qb == 0:
                            pvc = msc.tile([128, 2 * QB], F32, tag="pvc", name="pvc")
                            nc.vector.tensor_copy(out=pvc[:], in_=pv_ps[:])
                            nc.sync.dma_start(out=dbg["pv0"][:], in_=pvc[:])
                        # recip_approx_fast mis-reads PSUM at partition
                        # base 64; stage the denom row to SBUF first
                        nc.scalar.copy(dreg[:], pv_ps[64:65, :])
                        nc.vector.reciprocal_approx_fast(
                            out=rcp[:], in_=dreg[:])
                        if debug and qb == 0:
                            nc.sync.dma_start(out=dbg["rcp0"][:], in_=rcp[:])

                    def bcast():
                        nc.gpsimd.partition_broadcast(bc[:], rcp[:])

                    def norm():
                        for h in (0, 1):
                            nc.vector.tensor_tensor(
                                out=o2t[h * D:(h + 1) * D, :],
                                in0=pv_ps[0:64, h * QB:(h + 1) * QB],
                                in1=bc[0:64, h * QB:(h + 1) * QB],
                                op=ALU.mult)
                        if debug and qb == 0:
                            nc.sync.dma_start(out=dbg["o2t0"][:], in_=o2t[:])
                        return o2t
                    return recip, bcast, norm

                def oproj_round(qb, o2t, rnd, eng):
                    s_anchor = last_s[0]
                    po = sps.tile([128, 2 * QB], F32, tag="s", name="s")
                    for i in (0, 1):
                        qs = rnd * 2 + i
                        mm = nc.tensor.matmul(
                            po[:, i * C:(i + 1) * C],
                            lhsT=o2t[:, qs * 128:(qs + 1) * 128],
                            rhs=wo_s[:], start=True, stop=True)
                        if s_anchor is not None:
                            # keep the scheduler from hoisting this ahead of
                            # the S stream
                            add_dep_helper(mm.ins, s_anchor, False,
                                           "outproj after S stream")
                    ob = obp.tile([128, 2 * C], F32, tag="ob", name="ob")
                    nc.scalar.copy(ob[:], po[:])
                    r0 = qb * QB + rnd * 256
                    dst = out[r0:r0 + 256, :].rearrange("(u p) c -> p u c", u=2)
                    eng.dma_start(out=dst,
                                  in_=ob[:].rearrange("p (u c) -> p u c", u=2))

                # ---- main loop: PV runs one kc-step behind the exp ----
                pend_pv = []        # (p_tile, pkc) not yet emitted
                todo = {}           # kc slot -> [callables] (prev-qb epilogue)
                s_cur = s_mm(0, 0)
                for qb in range(NQB):
                    for kc in range(NKC):
                        if kc + 1 < NKC:
                            nxt = (qb, kc + 1)
                        elif qb + 1 < NQB:
                            nxt = (qb + 1, 0)
                        else:
                            nxt = None
                        s_next = s_mm(*nxt) if nxt else None
                        # emit pending PV pairs; a pair with pkc==0 must wait
                        # until the previous epilogue's PSUM reads (emitted at
                        # kc<=3) are in program order before it
                        budget = 2 if len(pend_pv) > 1 else 1
                        while pend_pv and budget > 0:
                            if qb > 0 and kc < 7 and pend_pv[0][1] == 0:
                                break
                            p_t, pkc = pend_pv.pop(0)
                            pv_mm(p_t, pkc)
                            budget -= 1
                        # previous q-block epilogue pieces
                        for fn in todo.pop(kc, ()):
                            fn()
                        # drip-feed remaining projections (qb=0 only)
                        n_items = 2 if (qb == 0 and kc < 8) else 1
                        for _ in range(n_items):
                            if pending_proj:
                                kind, idx = pending_proj.pop()
                                proj_v(idx) if kind == "v" else proj_qk(kind, idx)
                        if kc == 16 and qb < NQB - 1:
                            proj_qk("q", qb + 1)
                        if debug and qb == 1 and kc == 20:
                            nc.sync.dma_start(out=dbg["kt"][:], in_=kt[:])
                            nc.sync.dma_start(out=dbg["qt"][:], in_=qt[:])
                            nc.sync.dma_start(out=dbg["va"][:], in_=vaug[:])
                        # exp for this step; its PV queued for a later step
                        p = do_exp(s_cur, _dve_step(qb * NKC + kc))
                        if debug and qb == 0 and kc == 0:
                            nc.sync.dma_start(out=dbg["p00"][:], in_=p[:])
                        pend_pv.append((p, kc))
                        s_cur = s_next
                    if qb + 1 < NQB:
                        # schedule this q-block's epilogue into the next block
                        # (the PV tile is resolved at call time via pv_box —
                        # by kc=0 of qb+1 all 32 PV pairs of qb are... the
                        # last pair lands at kc=0; recip runs at kc=1)
                        pv_ps = pv_box[0]
                        recip, bcast, norm = make_epilogue(qb, pv_ps)
                        o2t_box = []
                        todo[1] = [recip]
                        todo[2] = [bcast]
                        todo[3] = [lambda n=norm, b=o2t_box: b.append(n())]
                        todo[10] = [lambda q=qb, b=o2t_box:
                                    oproj_round(q, b[0], 0, nc.sync)]
                        todo[13] = [lambda q=qb, b=o2t_box:
                                    oproj_round(q, b[0], 1, nc.gpsimd)]
                    else:
                        # tail: drain the last PV pair, then the epilogue
                        while pend_pv:
                            p_t, pkc = pend_pv.pop(0)
                            pv_mm(p_t, pkc)
                        recip, bcast, norm = make_epilogue(qb, pv_box[0])
                        recip()
                        bcast()
                        o2t = norm()
                        oproj_round(qb, o2t, 0, nc.sync)
                        oproj_round(qb, o2t, 1, nc.gpsimd)

    nc.compile()
    return nc


_NC_CACHE = {}


def _get_nc():
    if "nc" not in _NC_CACHE:
        _NC_CACHE["nc"] = build_nc()
    return _NC_CACHE["nc"]


def kernel(x, Wq, bq, Wk, bk, Wv, bv, Wo, bo):
    x = np.asarray(x, dtype=np.float32)
    bf = ml_dtypes.bfloat16
    nc = _get_nc()

    in_maps = []
    for c in range(8):
        b, p = c // 4, c % 4
        cs = slice(p * HD, (p + 1) * HD)
        in_maps.append({
            "xT": np.ascontiguousarray(x[b].T).astype(bf),
            "wq": np.ascontiguousarray(Wq[:, cs]).astype(bf),
            "wk": np.ascontiguousarray(Wk[:, cs]).astype(bf),
            "wv": np.ascontiguousarray(Wv[:, cs]).astype(bf),
            "wo": np.ascontiguousarray(Wo[cs, :]).astype(bf),
            "bq": np.asarray(bq[cs], np.float32).reshape(HD, 1).copy(),
            "bk": np.asarray(bk[cs], np.float32).reshape(HD, 1).copy(),
            "bv": np.asarray(bv[cs], np.float32).reshape(1, HD).astype(bf),
        })

    res = run_bass_kernel_spmd(nc, in_maps, core_ids=list(range(8)))

    out = np.zeros((2, N, C), np.float32)
    for c in range(8):
        out[c // 4] += res.results[c]["out"]
    out += np.asarray(bo, np.float32)[None, None, :]
    return out
